# revision 11
# baseline (speedup 1.0000x reference)
"""Trainium2 Bass kernel for nn_Attention_61443802137307.

Multi-head attention block:
    x_topo = x + topo_all_fea (if is_end)
    kv = x_topo @ kv_w.T ; q = x @ q_w.T (scaled by hd^-0.5, folded into q_w)
    attn = softmax(q k^T); out = (attn @ v) @ proj_w.T + proj_b

Sharding: data-parallel over batch (dim 0), 32 batches per core x 8 cores.

Per-core design (feature-major activations; no transposes):
  - host pre-transposes x / x+topo to [D, tok] layout, pre-transposes weights
  - projections in float32r (full-speed, ~1.5e-4): q_fm/k_fm feature-major
    bf16, v token-major bf16 with per-head ones columns (for softmax sums)
  - scoresT[k,q] per (b,head) via K=64 matmuls; exp on ACT (scale folded
    into q_w); av matmul contracts k, yielding out_fm[65,q] whose row 64 is
    the softmax denominator (from the ones column)
  - denominators: gathered by DMA, reciprocal via exp(-ln(s)) on ACT,
    broadcast across 64 partitions by a tiny K=8 matmul against a constant
    E matrix, applied by DVE multiply (which also relayouts heads into
    proj-ready [128,4,tok] chunks, f32r)
  - proj: 8 K=64 row-packed matmuls per token tile + bias add (DVE)
"""
import numpy as np

import concourse.bass as bass
import concourse.tile as tile
import concourse.mybir as mybir
from concourse import bacc
from concourse.bass_utils import run_bass_kernel_spmd
from contextlib import ExitStack

F32 = mybir.dt.float32
F32R = mybir.dt.float32r
BF16 = mybir.dt.bfloat16
AF = mybir.ActivationFunctionType

B, N, D = 256, 144, 512
H, HD = 8, 64
SCALE = HD ** -0.5
N_CORES = 8
BPC = B // N_CORES          # 32 batches per core
TOK = BPC * N               # 4608 tokens per core
GB = 4                      # batches per group
NG = BPC // GB              # 8 groups
GTOK = GB * N               # 576 tokens per group

_CACHE = {}


def _v_copyback_plan():
    """Token-tile -> per-batch v destination segments for one group.

    v projection runs on 128-row token tiles of the group's GTOK tokens;
    the av matmul needs per-batch tiles (vA: k 0:128, vB: k 128:144).
    Returns per tile index a list of (psum_row0, rows, b_local, dest, dest_row0)
    with dest in {"A", "B"}.
    """
    plan = []
    ntiles = (GTOK + 127) // 128
    for t in range(ntiles):
        r0 = 128 * t
        rows_in_tile = min(128, GTOK - r0)
        segs = []
        r = r0
        while r < r0 + rows_in_tile:
            b = r // N
            k = r - b * N
            if k < 128:
                seg = min(128 - k, r0 + rows_in_tile - r)
                segs.append((r - r0, seg, b, "A", k))
            else:
                seg = min(N - k, r0 + rows_in_tile - r)
                segs.append((r - r0, seg, b, "B", k - 128))
            r += seg
        plan.append(segs)
    return plan


def build():
    import os
    stop_after = os.environ.get("K_STOP_AFTER", "full")
    nc = bacc.Bacc("TRN2", target_bir_lowering=False, debug=False,
                   num_devices=N_CORES)

    xT = nc.dram_tensor("xT", [4, 128, TOK], F32R, kind="ExternalInput").ap()
    xkT = nc.dram_tensor("xkT", [4, 128, TOK], F32R, kind="ExternalInput").ap()
    kv_wT = nc.dram_tensor("kv_wT", [4, 128, 2 * D], F32R,
                           kind="ExternalInput").ap()
    q_wT = nc.dram_tensor("q_wT", [4, 128, D], F32R, kind="ExternalInput").ap()
    p_wT = nc.dram_tensor("p_wT", [4, 128, D], F32R, kind="ExternalInput").ap()
    p_b = nc.dram_tensor("p_b", [D], F32, kind="ExternalInput").ap()
    e_mat = nc.dram_tensor("e_mat", [8, D], F32R, kind="ExternalInput").ap()
    out = nc.dram_tensor("out", [TOK, D], F32, kind="ExternalOutput").ap()

    with tile.TileContext(nc) as tc, ExitStack() as ctx:
        singles = ctx.enter_context(tc.tile_pool(name="singles", bufs=1))
        xpool = ctx.enter_context(tc.tile_pool(name="xpool", bufs=2))
        qkpool = ctx.enter_context(tc.tile_pool(name="qkpool", bufs=2))
        vpool = ctx.enter_context(tc.tile_pool(name="vpool", bufs=8))
        epool = ctx.enter_context(tc.tile_pool(name="epool", bufs=3))
        aupool = ctx.enter_context(tc.tile_pool(name="aupool", bufs=1))
        acpool = ctx.enter_context(tc.tile_pool(name="acpool", bufs=1))
        spool = ctx.enter_context(tc.tile_pool(name="spool", bufs=2))
        opool = ctx.enter_context(tc.tile_pool(name="opool", bufs=3))
        ps_pj = ctx.enter_context(tc.tile_pool(name="ps_pj", bufs=2,
                                               space="PSUM"))
        ps_sc = ctx.enter_context(tc.tile_pool(name="ps_sc", bufs=1,
                                               space="PSUM"))
        ps_av = ctx.enter_context(tc.tile_pool(name="ps_av", bufs=2,
                                               space="PSUM"))

        # --- persistent weights ---
        kv_w_sb = singles.tile([128, 4, 2 * D], F32R)
        q_w_sb = singles.tile([128, 4, D], F32R)
        p_w_sb = singles.tile([128, 4, D], F32R)
        for kc in range(4):
            nc.sync.dma_start(kv_w_sb[:, kc, :], kv_wT[kc])
            nc.sync.dma_start(q_w_sb[:, kc, :], q_wT[kc])
            nc.sync.dma_start(p_w_sb[:, kc, :], p_wT[kc])

        bias_bc = singles.tile([128, D], F32)
        bias_src = bass.AP(tensor=p_b.tensor, offset=0, ap=[[0, 128], [1, D]])
        nc.gpsimd.dma_start(out=bias_bc[:], in_=bias_src)

        # E matrix: E_all[h', 64h:64h+64] = (h'==h); lhsT slices for the
        # recip broadcast matmuls. Host-built (compute engines need
        # 32-aligned partition bases, so memset per row is not possible).
        e_all = singles.tile([8, D], F32R)
        nc.sync.dma_start(e_all[:], e_mat)

        ntt = (GTOK + 127) // 128  # token tiles per group (5: 4x128 + 64)

        for g in range(NG):
            g0 = g * GTOK
            # --- stage inputs ---
            xg = xpool.tile([128, 4, GTOK], F32R, tag="xg")
            xkg = xpool.tile([128, 4, GTOK], F32R, tag="xkg")
            for kc in range(4):
                nc.sync.dma_start(xg[:, kc, :], xT[kc, :, g0:g0 + GTOK])
                nc.sync.dma_start(xkg[:, kc, :], xkT[kc, :, g0:g0 + GTOK])

            # --- q / k projections (feature-major, bf16 out) ---
            q_fm = qkpool.tile([128, 4, GTOK], BF16, tag="qfm")
            k_fm = qkpool.tile([128, 4, GTOK], BF16, tag="kfm")
            NT = GTOK // 2  # 288
            for dst, w_sb, src, w_off in (
                (q_fm, q_w_sb, xg, 0),
                (k_fm, kv_w_sb, xkg, 0),
            ):
                for fc in range(4):
                    for nt in range(2):
                        p = ps_pj.tile([128, NT], F32, tag="pj")
                        for kc in range(4):
                            nc.tensor.matmul(
                                p[:],
                                w_sb[:, kc, w_off + 128 * fc:w_off + 128 * fc + 128],
                                src[:, kc, NT * nt:NT * nt + NT],
                                start=(kc == 0), stop=(kc == 3))
                        nc.vector.tensor_copy(
                            dst[:, fc, NT * nt:NT * nt + NT], p[:])

            # --- v projection (token-major with ones cols, bf16) ---
            vA = [vpool.tile([128, H, 65], BF16, tag="vA", name=f"vA{g}_{i}")
                  for i in range(GB)]
            vB = [vpool.tile([16, H, 65], BF16, tag="vB", name=f"vB{g}_{i}")
                  for i in range(GB)]
            for b in range(GB):
                off = N * b
                for rows, k0, tgt in ((128, 0, vA[b]), (16, 128, vB[b])):
                    p = ps_pj.tile([128, D], F32, tag="pj")
                    for kc in range(4):
                        nc.tensor.matmul(
                            p[:rows, :],
                            xkg[:, kc, off + k0:off + k0 + rows],
                            kv_w_sb[:, kc, D:2 * D],
                            start=(kc == 0), stop=(kc == 3))
                    pv = p[:rows].rearrange("p (h d) -> p h d", h=H)
                    nc.scalar.copy(tgt[:, :, 0:64], pv)
                nc.vector.memset(vA[b][:, :, 64:65], 1.0)
                nc.vector.memset(vB[b][:, :, 64:65], 1.0)

            if stop_after == "qkv":
                continue
            # --- attention per batch ---
            attn_u = aupool.tile([65, H, GTOK], F32, tag="au")
            for b in range(GB):
                off = N * b
                for pr in range(4):  # head pairs
                    sc = ps_sc.tile([128, 1024], F32, tag="sc")
                    for j in range(2):
                        h = 2 * pr + j
                        fc, r0 = h // 2, (h % 2) * 64
                        lhs1 = k_fm[r0:r0 + 64, fc, off:off + 128]
                        lhs2 = k_fm[r0:r0 + 64, fc, off + 128:off + 144]
                        rhs = q_fm[r0:r0 + 64, fc, off:off + 144]
                        nc.tensor.matmul(sc[:, 512 * j:512 * j + 144],
                                         lhs1, rhs, start=True, stop=True)
                        nc.tensor.matmul(sc[0:16, 512 * j + 144:512 * j + 288],
                                         lhs2, rhs, start=True, stop=True)
                    e1 = epool.tile([128, 2, 144], BF16, tag="e1")
                    e2 = epool.tile([16, 2, 144], BF16, tag="e2")
                    sc1 = sc[:].rearrange("p (j c) -> p j c", j=2)[:, :, 0:144]
                    sc2 = sc[0:16].rearrange("p (j c) -> p j c", j=2)[:, :, 144:288]
                    nc.scalar.activation(e1[:], sc1, AF.Exp)
                    nc.scalar.activation(e2[:], sc2, AF.Exp)

                    av = ps_av.tile([128, 1024], F32, tag="av")
                    for j in range(2):
                        h = 2 * pr + j
                        nc.tensor.matmul(av[0:65, 512 * j:512 * j + 144],
                                         vA[b][:, h, :], e1[:, j, :],
                                         start=True, stop=False)
                        nc.tensor.matmul(av[0:65, 512 * j:512 * j + 144],
                                         vB[b][:, h, :], e2[:, j, :],
                                         start=False, stop=True)
                    avv = av[0:65].rearrange("p (j c) -> p j c", j=2)[:, :, 0:144]
                    nc.scalar.copy(
                        attn_u[:, 2 * pr:2 * pr + 2, off:off + 144], avv)

            if stop_after == "attn":
                continue
            # --- deferred softmax normalization + head relayout ---
            sums_g = spool.tile([8, GTOK], F32, tag="sums")
            for h in range(H):
                nc.sync.dma_start(sums_g[h:h + 1, :], attn_u[64:65, h, :])
            lnsum = spool.tile([8, GTOK], F32, tag="lnsum")
            recip_g = spool.tile([8, GTOK], F32R, tag="recip")
            nc.scalar.activation(lnsum[:], sums_g[:], AF.Ln)
            nc.scalar.activation(recip_g[:], lnsum[:], AF.Exp, scale=-1.0)

            attn_c = acpool.tile([128, 4, GTOK], F32R, tag="ac")
            NB = GTOK // 2  # 288
            for h in range(H):
                fc, r0 = h // 2, (h % 2) * 64
                for nt in range(2):
                    bc = ps_pj.tile([64, NB], F32, tag="pj")
                    nc.tensor.matmul(bc[:], e_all[:, 64 * h:64 * h + 64],
                                     recip_g[:, NB * nt:NB * nt + NB],
                                     start=True, stop=True)
                    nc.vector.tensor_mul(
                        attn_c[r0:r0 + 64, fc, NB * nt:NB * nt + NB],
                        attn_u[0:64, h, NB * nt:NB * nt + NB],
                        bc[:])

            if stop_after == "norm":
                continue
            # --- output projection + bias ---
            for t in range(ntt):
                rows = min(128, GTOK - 128 * t)
                p = ps_av.tile([128, D], F32, tag="av")
                for fc in range(4):
                    nc.tensor.matmul(
                        p[:rows, :],
                        attn_c[:, fc, 128 * t:128 * t + rows],
                        p_w_sb[:, fc, :],
                        start=(fc == 0), stop=(fc == 3))
                o_sb = opool.tile([128, D], F32, tag="osb")
                nc.vector.tensor_add(o_sb[:rows, :], p[:rows, :],
                                     bias_bc[:rows, :])
                nc.sync.dma_start(
                    out[g0 + 128 * t:g0 + 128 * t + rows, :], o_sb[:rows, :])

    nc.compile()
    return nc


def _get_nc():
    if "nc" not in _CACHE:
        _CACHE["nc"] = build()
    return _CACHE["nc"]


def _prep_core_inputs(x, xk, kv_wT, q_wT, p_wT, p_b):
    """x, xk: [BPC, N, D] fp32 for one core."""
    def fm(a):  # [tok, D] -> [4, 128, tok] feature-major chunks
        t = np.ascontiguousarray(a.reshape(BPC * N, D).T)  # [D, tok]
        return t.reshape(4, 128, BPC * N)

    e_mat = np.zeros((8, D), dtype=np.float32)
    for h in range(H):
        e_mat[h, 64 * h:64 * h + 64] = 1.0
    return {
        "xT": fm(x), "xkT": fm(xk),
        "kv_wT": kv_wT, "q_wT": q_wT, "p_wT": p_wT, "p_b": p_b,
        "e_mat": e_mat,
    }


def kernel(x, topo_all_fea, kv_w, q_w, proj_w, proj_b, is_end):
    x = np.asarray(x, dtype=np.float32)
    topo = np.asarray(topo_all_fea, dtype=np.float32)
    kv_w = np.asarray(kv_w, dtype=np.float32)
    q_w = np.asarray(q_w, dtype=np.float32)
    proj_w = np.asarray(proj_w, dtype=np.float32)
    proj_b = np.asarray(proj_b, dtype=np.float32)
    end = bool(np.asarray(is_end).item()) if not isinstance(is_end, bool) \
        else is_end

    xk = x + topo if end else x

    kv_wT = np.ascontiguousarray(kv_w.T).reshape(4, 128, 2 * D)
    q_wT = np.ascontiguousarray(q_w.T * SCALE).reshape(4, 128, D)
    p_wT = np.ascontiguousarray(proj_w.T).reshape(4, 128, D)

    nc = _get_nc()
    in_maps = [
        _prep_core_inputs(x[c * BPC:(c + 1) * BPC],
                          xk[c * BPC:(c + 1) * BPC],
                          kv_wT, q_wT, p_wT, proj_b)
        for c in range(N_CORES)
    ]
    res = run_bass_kernel_spmd(nc, in_maps, core_ids=list(range(N_CORES)))
    outs = [res.results[c]["out"].reshape(BPC, N, D) for c in range(N_CORES)]
    return np.concatenate(outs, axis=0)


# revision 13
# speedup vs baseline: 1.3673x; 1.3673x over previous
"""Trainium2 Bass kernel for nn_Attention_61443802137307.

Multi-head attention block:
    x_topo = x + topo_all_fea (if is_end)
    kv = x_topo @ kv_w.T ; q = x @ q_w.T (scaled by hd^-0.5, folded into q_w)
    attn = softmax(q k^T); out = (attn @ v) @ proj_w.T + proj_b

Sharding: data-parallel over batch (dim 0), 32 batches per core x 8 cores.

Per-core design (feature-major activations; no transposes):
  - host pre-transposes x / x+topo to [D, tok] layout, pre-transposes weights
  - projections in float32r (full-speed, ~1.5e-4): q_fm/k_fm feature-major
    bf16, v token-major bf16 with per-head ones columns (for softmax sums)
  - scoresT[k,q] per (b,head) via K=64 matmuls; exp on ACT (scale folded
    into q_w); av matmul contracts k, yielding out_fm[65,q] whose row 64 is
    the softmax denominator (from the ones column)
  - denominators: gathered by DMA, reciprocal via exp(-ln(s)) on ACT,
    broadcast across 64 partitions by a tiny K=8 matmul against a constant
    E matrix, applied by DVE multiply (which also relayouts heads into
    proj-ready [128,4,tok] chunks, f32r)
  - proj: 8 K=64 row-packed matmuls per token tile + bias add (DVE)
"""
import numpy as np

import concourse.bass as bass
import concourse.tile as tile
import concourse.mybir as mybir
from concourse import bacc
from concourse.bass_utils import run_bass_kernel_spmd
from contextlib import ExitStack

F32 = mybir.dt.float32
F32R = mybir.dt.float32r
BF16 = mybir.dt.bfloat16
AF = mybir.ActivationFunctionType

B, N, D = 256, 144, 512
H, HD = 8, 64
SCALE = HD ** -0.5
N_CORES = 8
BPC = B // N_CORES          # 32 batches per core
TOK = BPC * N               # 4608 tokens per core
GB = 4                      # batches per group
NG = BPC // GB              # 8 groups
GTOK = GB * N               # 576 tokens per group

_CACHE = {}


def _v_copyback_plan():
    """Token-tile -> per-batch v destination segments for one group.

    v projection runs on 128-row token tiles of the group's GTOK tokens;
    the av matmul needs per-batch tiles (vA: k 0:128, vB: k 128:144).
    Returns per tile index a list of (psum_row0, rows, b_local, dest, dest_row0)
    with dest in {"A", "B"}.
    """
    plan = []
    ntiles = (GTOK + 127) // 128
    for t in range(ntiles):
        r0 = 128 * t
        rows_in_tile = min(128, GTOK - r0)
        segs = []
        r = r0
        while r < r0 + rows_in_tile:
            b = r // N
            k = r - b * N
            if k < 128:
                seg = min(128 - k, r0 + rows_in_tile - r)
                segs.append((r - r0, seg, b, "A", k))
            else:
                seg = min(N - k, r0 + rows_in_tile - r)
                segs.append((r - r0, seg, b, "B", k - 128))
            r += seg
        plan.append(segs)
    return plan


def build():
    import os
    stop_after = os.environ.get("K_STOP_AFTER", "full")
    nc = bacc.Bacc("TRN2", target_bir_lowering=False, debug=False,
                   num_devices=N_CORES)

    xT = nc.dram_tensor("xT", [4, 128, TOK], F32R, kind="ExternalInput").ap()
    xkT = nc.dram_tensor("xkT", [4, 128, TOK], F32R, kind="ExternalInput").ap()
    kv_wT = nc.dram_tensor("kv_wT", [4, 128, 2 * D], F32R,
                           kind="ExternalInput").ap()
    q_wT = nc.dram_tensor("q_wT", [4, 128, D], F32R, kind="ExternalInput").ap()
    p_wT = nc.dram_tensor("p_wT", [4, 128, D], F32R, kind="ExternalInput").ap()
    p_b = nc.dram_tensor("p_b", [D], F32, kind="ExternalInput").ap()
    e_mat = nc.dram_tensor("e_mat", [8, D], F32R, kind="ExternalInput").ap()
    out = nc.dram_tensor("out", [TOK, D], F32, kind="ExternalOutput").ap()

    with tile.TileContext(nc) as tc, ExitStack() as ctx:
        import os as _os
        def _bufs(name, d):
            return int(_os.environ.get(f"K_BUFS_{name}", d))
        singles = ctx.enter_context(tc.tile_pool(name="singles", bufs=1))
        xpool = ctx.enter_context(tc.tile_pool(name="xpool", bufs=_bufs("X", 2)))
        qkpool = ctx.enter_context(tc.tile_pool(name="qkpool", bufs=_bufs("QK", 2)))
        vpool = ctx.enter_context(tc.tile_pool(name="vpool", bufs=_bufs("V", 8)))
        epool = ctx.enter_context(tc.tile_pool(name="epool", bufs=_bufs("E", 3)))
        aupool = ctx.enter_context(tc.tile_pool(name="aupool", bufs=_bufs("AU", 2)))
        acpool = ctx.enter_context(tc.tile_pool(name="acpool", bufs=_bufs("AC", 2)))
        spool = ctx.enter_context(tc.tile_pool(name="spool", bufs=2))
        opool = ctx.enter_context(tc.tile_pool(name="opool", bufs=_bufs("O", 3)))
        ps_pj = ctx.enter_context(tc.tile_pool(name="ps_pj", bufs=_bufs("PJ", 2),
                                               space="PSUM"))
        ps_sc = ctx.enter_context(tc.tile_pool(name="ps_sc", bufs=_bufs("SC", 1),
                                               space="PSUM"))
        ps_av = ctx.enter_context(tc.tile_pool(name="ps_av", bufs=_bufs("AV", 2),
                                               space="PSUM"))

        # --- persistent weights ---
        kv_w_sb = singles.tile([128, 4, 2 * D], F32R)
        q_w_sb = singles.tile([128, 4, D], F32R)
        p_w_sb = singles.tile([128, 4, D], F32R)
        for kc in range(4):
            nc.sync.dma_start(kv_w_sb[:, kc, :], kv_wT[kc])
            nc.sync.dma_start(q_w_sb[:, kc, :], q_wT[kc])
            nc.sync.dma_start(p_w_sb[:, kc, :], p_wT[kc])

        bias_bc = singles.tile([128, D], F32)
        bias_src = bass.AP(tensor=p_b.tensor, offset=0, ap=[[0, 128], [1, D]])
        nc.gpsimd.dma_start(out=bias_bc[:], in_=bias_src)

        # E matrix: E_all[h', 64h:64h+64] = (h'==h); lhsT slices for the
        # recip broadcast matmuls. Host-built (compute engines need
        # 32-aligned partition bases, so memset per row is not possible).
        e_all = singles.tile([8, D], F32R)
        nc.sync.dma_start(e_all[:], e_mat)

        ntt = (GTOK + 127) // 128  # token tiles per group (5: 4x128 + 64)

        def phase_A(g):
            """Stage inputs + q/k/v projections for group g."""
            g0 = g * GTOK
            xg = xpool.tile([128, 4, GTOK], F32R, tag="xg", name=f"xg{g}")
            xkg = xpool.tile([128, 4, GTOK], F32R, tag="xkg", name=f"xkg{g}")
            for kc in range(4):
                nc.sync.dma_start(xg[:, kc, :], xT[kc, :, g0:g0 + GTOK])
                nc.sync.dma_start(xkg[:, kc, :], xkT[kc, :, g0:g0 + GTOK])

            q_fm = qkpool.tile([128, 4, GTOK], BF16, tag="qfm", name=f"qfm{g}")
            k_fm = qkpool.tile([128, 4, GTOK], BF16, tag="kfm", name=f"kfm{g}")
            NT = GTOK // 2  # 288
            for dst, w_sb, src, w_off in (
                (q_fm, q_w_sb, xg, 0),
                (k_fm, kv_w_sb, xkg, 0),
            ):
                for fc in range(4):
                    for nt in range(2):
                        p = ps_pj.tile([128, NT], F32, tag="pj", name=f"pjq{g}")
                        for kc in range(4):
                            nc.tensor.matmul(
                                p[:],
                                w_sb[:, kc, w_off + 128 * fc:w_off + 128 * fc + 128],
                                src[:, kc, NT * nt:NT * nt + NT],
                                start=(kc == 0), stop=(kc == 3))
                        nc.vector.tensor_copy(
                            dst[:, fc, NT * nt:NT * nt + NT], p[:])

            vA = [vpool.tile([128, H, 65], BF16, tag="vA", name=f"vA{g}_{i}")
                  for i in range(GB)]
            vB = [vpool.tile([16, H, 65], BF16, tag="vB", name=f"vB{g}_{i}")
                  for i in range(GB)]
            for b in range(GB):
                off = N * b
                for rows, k0, tgt in ((128, 0, vA[b]), (16, 128, vB[b])):
                    p = ps_pj.tile([128, D], F32, tag="pj", name=f"pjv{g}")
                    for kc in range(4):
                        nc.tensor.matmul(
                            p[:rows, :],
                            xkg[:, kc, off + k0:off + k0 + rows],
                            kv_w_sb[:, kc, D:2 * D],
                            start=(kc == 0), stop=(kc == 3))
                    pv = p[:rows].rearrange("p (h d) -> p h d", h=H)
                    nc.scalar.copy(tgt[:, :, 0:64], pv)
                nc.vector.memset(vA[b][:, :, 64:65], 1.0)
                nc.vector.memset(vB[b][:, :, 64:65], 1.0)
            return q_fm, k_fm, vA, vB

        def phase_B(g, q_fm, k_fm, vA, vB):
            """Attention for group g -> attn_u [65, H, GTOK]."""
            attn_u = aupool.tile([65, H, GTOK], F32, tag="au", name=f"au{g}")
            for b in range(GB):
                off = N * b
                for pr in range(4):  # head pairs
                    sc = ps_sc.tile([128, 1024], F32, tag="sc", name=f"sc{g}")
                    for j in range(2):
                        h = 2 * pr + j
                        fc, r0 = h // 2, (h % 2) * 64
                        lhs1 = k_fm[r0:r0 + 64, fc, off:off + 128]
                        lhs2 = k_fm[r0:r0 + 64, fc, off + 128:off + 144]
                        rhs = q_fm[r0:r0 + 64, fc, off:off + 144]
                        nc.tensor.matmul(sc[:, 512 * j:512 * j + 144],
                                         lhs1, rhs, start=True, stop=True)
                        nc.tensor.matmul(sc[0:16, 512 * j + 144:512 * j + 288],
                                         lhs2, rhs, start=True, stop=True)
                    e1 = epool.tile([128, 2, 144], BF16, tag="e1", name=f"e1_{g}")
                    e2 = epool.tile([16, 2, 144], BF16, tag="e2", name=f"e2_{g}")
                    sc1 = sc[:].rearrange("p (j c) -> p j c", j=2)[:, :, 0:144]
                    sc2 = sc[0:16].rearrange("p (j c) -> p j c", j=2)[:, :, 144:288]
                    nc.scalar.activation(e1[:], sc1, AF.Exp)
                    nc.scalar.activation(e2[:], sc2, AF.Exp)

                    av = ps_av.tile([128, 1024], F32, tag="av", name=f"av{g}")
                    for j in range(2):
                        h = 2 * pr + j
                        nc.tensor.matmul(av[0:65, 512 * j:512 * j + 144],
                                         vA[b][:, h, :], e1[:, j, :],
                                         start=True, stop=False)
                        nc.tensor.matmul(av[0:65, 512 * j:512 * j + 144],
                                         vB[b][:, h, :], e2[:, j, :],
                                         start=False, stop=True)
                    avv = av[0:65].rearrange("p (j c) -> p j c", j=2)[:, :, 0:144]
                    nc.scalar.copy(
                        attn_u[:, 2 * pr:2 * pr + 2, off:off + 144], avv)
            return attn_u

        def phase_C(g, attn_u):
            """Softmax normalization + head relayout + output projection."""
            g0 = g * GTOK
            sums_g = spool.tile([8, GTOK], F32, tag="sums", name=f"sums{g}")
            for h in range(H):
                nc.sync.dma_start(sums_g[h:h + 1, :], attn_u[64:65, h, :])
            lnsum = spool.tile([8, GTOK], F32, tag="lnsum", name=f"ln{g}")
            recip_g = spool.tile([8, GTOK], F32R, tag="recip", name=f"rc{g}")
            nc.scalar.activation(lnsum[:], sums_g[:], AF.Ln)
            nc.scalar.activation(recip_g[:], lnsum[:], AF.Exp, scale=-1.0)

            attn_c = acpool.tile([128, 4, GTOK], F32R, tag="ac", name=f"ac{g}")
            NB = GTOK // 2  # 288
            for h in range(H):
                fc, r0 = h // 2, (h % 2) * 64
                for nt in range(2):
                    bc = ps_pj.tile([64, NB], F32, tag="pj", name=f"pjb{g}")
                    nc.tensor.matmul(bc[:], e_all[:, 64 * h:64 * h + 64],
                                     recip_g[:, NB * nt:NB * nt + NB],
                                     start=True, stop=True)
                    nc.vector.tensor_mul(
                        attn_c[r0:r0 + 64, fc, NB * nt:NB * nt + NB],
                        attn_u[0:64, h, NB * nt:NB * nt + NB],
                        bc[:])

            for t in range(ntt):
                rows = min(128, GTOK - 128 * t)
                p = ps_av.tile([128, D], F32, tag="av", name=f"avp{g}")
                for fc in range(4):
                    nc.tensor.matmul(
                        p[:rows, :],
                        attn_c[:, fc, 128 * t:128 * t + rows],
                        p_w_sb[:, fc, :],
                        start=(fc == 0), stop=(fc == 3))
                o_sb = opool.tile([128, D], F32, tag="osb", name=f"osb{g}")
                nc.vector.tensor_add(o_sb[:rows, :], p[:rows, :],
                                     bias_bc[:rows, :])
                nc.sync.dma_start(
                    out[g0 + 128 * t:g0 + 128 * t + rows, :], o_sb[:rows, :])

        # software-pipelined emission: A(g), C(g-1), B(g)
        pend = None
        for g in range(NG):
            qkv = phase_A(g)
            if stop_after == "qkv":
                continue
            if pend is not None:
                phase_C(pend[0], pend[1])
            attn_u = phase_B(g, *qkv)
            if stop_after in ("attn", "norm"):
                continue
            pend = (g, attn_u)
        if pend is not None:
            phase_C(pend[0], pend[1])

    nc.compile()
    return nc


def _get_nc():
    if "nc" not in _CACHE:
        _CACHE["nc"] = build()
    return _CACHE["nc"]


def _prep_core_inputs(x, xk, kv_wT, q_wT, p_wT, p_b):
    """x, xk: [BPC, N, D] fp32 for one core."""
    def fm(a):  # [tok, D] -> [4, 128, tok] feature-major chunks
        t = np.ascontiguousarray(a.reshape(BPC * N, D).T)  # [D, tok]
        return t.reshape(4, 128, BPC * N)

    e_mat = np.zeros((8, D), dtype=np.float32)
    for h in range(H):
        e_mat[h, 64 * h:64 * h + 64] = 1.0
    return {
        "xT": fm(x), "xkT": fm(xk),
        "kv_wT": kv_wT, "q_wT": q_wT, "p_wT": p_wT, "p_b": p_b,
        "e_mat": e_mat,
    }


def kernel(x, topo_all_fea, kv_w, q_w, proj_w, proj_b, is_end):
    x = np.asarray(x, dtype=np.float32)
    topo = np.asarray(topo_all_fea, dtype=np.float32)
    kv_w = np.asarray(kv_w, dtype=np.float32)
    q_w = np.asarray(q_w, dtype=np.float32)
    proj_w = np.asarray(proj_w, dtype=np.float32)
    proj_b = np.asarray(proj_b, dtype=np.float32)
    end = bool(np.asarray(is_end).item()) if not isinstance(is_end, bool) \
        else is_end

    xk = x + topo if end else x

    kv_wT = np.ascontiguousarray(kv_w.T).reshape(4, 128, 2 * D)
    q_wT = np.ascontiguousarray(q_w.T * SCALE).reshape(4, 128, D)
    p_wT = np.ascontiguousarray(proj_w.T).reshape(4, 128, D)

    nc = _get_nc()
    in_maps = [
        _prep_core_inputs(x[c * BPC:(c + 1) * BPC],
                          xk[c * BPC:(c + 1) * BPC],
                          kv_wT, q_wT, p_wT, proj_b)
        for c in range(N_CORES)
    ]
    res = run_bass_kernel_spmd(nc, in_maps, core_ids=list(range(N_CORES)))
    outs = [res.results[c]["out"].reshape(BPC, N, D) for c in range(N_CORES)]
    return np.concatenate(outs, axis=0)


# revision 19
# speedup vs baseline: 1.4794x; 1.0819x over previous
"""Trainium2 Bass kernel for nn_Attention_61443802137307.

Multi-head attention block:
    x_topo = x + topo_all_fea (if is_end)
    kv = x_topo @ kv_w.T ; q = x @ q_w.T (scaled by hd^-0.5, folded into q_w)
    attn = softmax(q k^T); out = (attn @ v) @ proj_w.T + proj_b

Sharding: data-parallel over batch (dim 0), 32 batches per core x 8 cores.

Per-core design (feature-major activations; no transposes):
  - host pre-transposes x / x+topo to [D, tok] layout, pre-transposes weights
  - projections in float32r (full-speed, ~1.5e-4): q_fm/k_fm feature-major
    bf16, v token-major bf16 with per-head ones columns (for softmax sums)
  - scoresT[k,q] per (b,head) via K=64 matmuls; exp on ACT (scale folded
    into q_w); av matmul contracts k, yielding out_fm[65,q] whose row 64 is
    the softmax denominator (from the ones column)
  - denominators: gathered by DMA, reciprocal via exp(-ln(s)) on ACT,
    broadcast across 64 partitions by a tiny K=8 matmul against a constant
    E matrix, applied by DVE multiply (which also relayouts heads into
    proj-ready [128,4,tok] chunks, f32r)
  - proj: 8 K=64 row-packed matmuls per token tile + bias add (DVE)
"""
import numpy as np

import concourse.bass as bass
import concourse.tile as tile
import concourse.mybir as mybir
from concourse import bacc
from concourse.bass_utils import run_bass_kernel_spmd
from contextlib import ExitStack

F32 = mybir.dt.float32
F32R = mybir.dt.float32r
BF16 = mybir.dt.bfloat16
AF = mybir.ActivationFunctionType

B, N, D = 256, 144, 512
H, HD = 8, 64
SCALE = HD ** -0.5
N_CORES = 8
BPC = B // N_CORES          # 32 batches per core
TOK = BPC * N               # 4608 tokens per core
GB = 4                      # batches per group
NG = BPC // GB              # 8 groups
GTOK = GB * N               # 576 tokens per group

_CACHE = {}


def _v_copyback_plan():
    """Token-tile -> per-batch v destination segments for one group.

    v projection runs on 128-row token tiles of the group's GTOK tokens;
    the av matmul needs per-batch tiles (vA: k 0:128, vB: k 128:144).
    Returns per tile index a list of (psum_row0, rows, b_local, dest, dest_row0)
    with dest in {"A", "B"}.
    """
    plan = []
    ntiles = (GTOK + 127) // 128
    for t in range(ntiles):
        r0 = 128 * t
        rows_in_tile = min(128, GTOK - r0)
        segs = []
        r = r0
        while r < r0 + rows_in_tile:
            b = r // N
            k = r - b * N
            if k < 128:
                seg = min(128 - k, r0 + rows_in_tile - r)
                segs.append((r - r0, seg, b, "A", k))
            else:
                seg = min(N - k, r0 + rows_in_tile - r)
                segs.append((r - r0, seg, b, "B", k - 128))
            r += seg
        plan.append(segs)
    return plan


def build():
    import os
    stop_after = os.environ.get("K_STOP_AFTER", "full")
    nc = bacc.Bacc("TRN2", target_bir_lowering=False, debug=False,
                   num_devices=N_CORES)

    xT = nc.dram_tensor("xT", [4, 128, TOK], F32R, kind="ExternalInput").ap()
    xkT = nc.dram_tensor("xkT", [4, 128, TOK], F32R, kind="ExternalInput").ap()
    kv_wT = nc.dram_tensor("kv_wT", [4, 128, 2 * D], F32R,
                           kind="ExternalInput").ap()
    q_wT = nc.dram_tensor("q_wT", [4, 128, D], F32R, kind="ExternalInput").ap()
    p_wT = nc.dram_tensor("p_wT", [4, 128, D], F32R, kind="ExternalInput").ap()
    p_b = nc.dram_tensor("p_b", [D], F32, kind="ExternalInput").ap()
    e_mat = nc.dram_tensor("e_mat", [8, D], F32R, kind="ExternalInput").ap()
    out = nc.dram_tensor("out", [TOK, D], F32, kind="ExternalOutput").ap()

    with tile.TileContext(nc) as tc, ExitStack() as ctx:
        import os as _os
        def _bufs(name, d):
            return int(_os.environ.get(f"K_BUFS_{name}", d))
        singles = ctx.enter_context(tc.tile_pool(name="singles", bufs=1))
        xpool = ctx.enter_context(tc.tile_pool(name="xpool", bufs=_bufs("X", 2)))
        qkpool = ctx.enter_context(tc.tile_pool(name="qkpool", bufs=_bufs("QK", 2)))
        vpool = ctx.enter_context(tc.tile_pool(name="vpool", bufs=_bufs("V", 8)))
        epool = ctx.enter_context(tc.tile_pool(name="epool", bufs=_bufs("E", 3)))
        aupool = ctx.enter_context(tc.tile_pool(name="aupool", bufs=_bufs("AU", 2)))
        acpool = ctx.enter_context(tc.tile_pool(name="acpool", bufs=_bufs("AC", 2)))
        spool = ctx.enter_context(tc.tile_pool(name="spool", bufs=2))
        opool = ctx.enter_context(tc.tile_pool(name="opool", bufs=_bufs("O", 3)))
        ps_pj = ctx.enter_context(tc.tile_pool(name="ps_pj", bufs=_bufs("PJ", 2),
                                               space="PSUM"))
        ps_sc = ctx.enter_context(tc.tile_pool(name="ps_sc", bufs=_bufs("SC", 1),
                                               space="PSUM"))
        ps_av = ctx.enter_context(tc.tile_pool(name="ps_av", bufs=_bufs("AV", 2),
                                               space="PSUM"))

        # --- persistent weights ---
        kv_w_sb = singles.tile([128, 4, 2 * D], F32R)
        q_w_sb = singles.tile([128, 4, D], F32R)
        p_w_sb = singles.tile([128, 4, D], F32R)
        for kc in range(4):
            nc.sync.dma_start(kv_w_sb[:, kc, :], kv_wT[kc])
            nc.sync.dma_start(q_w_sb[:, kc, :], q_wT[kc])
            nc.sync.dma_start(p_w_sb[:, kc, :], p_wT[kc])

        bias_bc = singles.tile([128, D], F32)
        bias_src = bass.AP(tensor=p_b.tensor, offset=0, ap=[[0, 128], [1, D]])
        nc.gpsimd.dma_start(out=bias_bc[:], in_=bias_src)

        # E matrix: E_all[h', 64h:64h+64] = (h'==h); lhsT slices for the
        # recip broadcast matmuls. Host-built (compute engines need
        # 32-aligned partition bases, so memset per row is not possible).
        e_all = singles.tile([8, D], F32R)
        nc.sync.dma_start(e_all[:], e_mat)

        ntt = (GTOK + 127) // 128  # token tiles per group (5: 4x128 + 64)

        def phase_A_load(g):
            g0 = g * GTOK
            xg = xpool.tile([128, 4, GTOK], F32R, tag="xg", name=f"xg{g}")
            xkg = xpool.tile([128, 4, GTOK], F32R, tag="xkg", name=f"xkg{g}")
            for kc in range(4):
                nc.sync.dma_start(xg[:, kc, :], xT[kc, :, g0:g0 + GTOK])
                nc.sync.dma_start(xkg[:, kc, :], xkT[kc, :, g0:g0 + GTOK])
            q_fm = qkpool.tile([128, 4, GTOK], BF16, tag="qfm", name=f"qfm{g}")
            k_fm = qkpool.tile([128, 4, GB * 256], BF16, tag="kfm",
                               name=f"kfm{g}")
            # zero the per-batch k padding (cols 144:256 of each 256 block)
            kv4 = k_fm[:].rearrange("p f (b c) -> p f b c", c=256)
            nc.gpsimd.memset(kv4[:, :, :, 144:256], 0.0)
            vA = [vpool.tile([128, H, 65], BF16, tag="vA", name=f"vA{g}_{i}")
                  for i in range(GB)]
            vB = [vpool.tile([128, H, 65], BF16, tag="vB", name=f"vB{g}_{i}")
                  for i in range(GB)]
            return (xg, xkg, q_fm, k_fm, vA, vB)

        def phase_A_part(g, i, st):
            xg, xkg, q_fm, k_fm, vA, vB = st
            NT = GTOK // 2  # 288
            fc = i
            for isq, (dst, w_sb, src) in enumerate(
                    ((q_fm, q_w_sb, xg), (k_fm, kv_w_sb, xkg))):
                for nt in range(2):
                    p = ps_pj.tile([128, NT], F32, tag="pj", name=f"pjq{g}_{i}")
                    for kc in range(4):
                        nc.tensor.matmul(
                            p[:],
                            w_sb[:, kc, 128 * fc:128 * fc + 128],
                            src[:, kc, NT * nt:NT * nt + NT],
                            start=(kc == 0), stop=(kc == 3))
                    if isq == 0:
                        nc.vector.tensor_copy(
                            dst[:, fc, NT * nt:NT * nt + NT], p[:])
                    else:
                        # k: tokens 288*nt..288*nt+288 = 2 batches of 144,
                        # destination blocks are 256 apart
                        dview = dst[:].rearrange("p f (b c) -> p f b c", c=256)
                        nc.vector.tensor_copy(
                            dview[:, fc, 2 * nt:2 * nt + 2, 0:144],
                            p[:].rearrange("p (b c) -> p b c", b=2))
            b = i
            off = N * b
            nc.gpsimd.memset(vB[b][:], 0.0)
            for rows, k0, tgt in ((128, 0, vA[b]), (16, 128, vB[b])):
                p = ps_pj.tile([128, D], F32, tag="pj", name=f"pjv{g}_{i}")
                for kc in range(4):
                    nc.tensor.matmul(
                        p[:rows, :],
                        xkg[:, kc, off + k0:off + k0 + rows],
                        kv_w_sb[:, kc, D:2 * D],
                        start=(kc == 0), stop=(kc == 3))
                pv = p[:rows].rearrange("p (h d) -> p h d", h=H)
                nc.scalar.copy(tgt[0:rows, :, 0:64], pv)
            nc.gpsimd.memset(vA[b][:, :, 64:65], 1.0)
            nc.gpsimd.memset(vB[b][0:16, :, 64:65], 1.0)

        def phase_B_start(g):
            return aupool.tile([65, H, GTOK], F32, tag="au", name=f"au{g}")

        def phase_B_part(g, b, st, attn_u):
            _, _, q_fm, k_fm, vA, vB = st
            off = N * b
            koff = 256 * b
            for pr in range(4):  # head pairs
                sc = ps_sc.tile([128, 1024], F32, tag="sc", name=f"sc{g}_{b}")
                for j in range(2):
                    h = 2 * pr + j
                    fc, r0 = h // 2, (h % 2) * 64
                    lhs1 = k_fm[r0:r0 + 64, fc, koff:koff + 128]
                    lhs2 = k_fm[r0:r0 + 64, fc, koff + 128:koff + 256]
                    rhs = q_fm[r0:r0 + 64, fc, off:off + 144]
                    nc.tensor.matmul(sc[:, 512 * j:512 * j + 144],
                                     lhs1, rhs, start=True, stop=True)
                    nc.tensor.matmul(sc[:, 512 * j + 144:512 * j + 288],
                                     lhs2, rhs, start=True, stop=True)
                # single exp over [128, 2, 288]: cols 144:288 rows 16:128 are
                # stale psum (bounded old scores) -> harmless garbage in e1,
                # never consumed downstream.
                e1 = epool.tile([128, 2, 288], BF16, tag="e1", name=f"e1_{g}{b}")
                sc1 = sc[:].rearrange("p (j c) -> p j c", j=2)[:, :, 0:288]
                nc.scalar.activation(e1[:], sc1, AF.Exp)

                av = ps_av.tile([128, 512], F32, tag="av", name=f"av{g}_{b}")
                for j in range(2):
                    h = 2 * pr + j
                    nc.tensor.matmul(av[0:65, 256 * j:256 * j + 144],
                                     vA[b][:, h, :], e1[:, j, 0:144],
                                     start=True, stop=False)
                    nc.tensor.matmul(av[0:65, 256 * j:256 * j + 144],
                                     vB[b][:, h, :], e1[:, j, 144:288],
                                     start=False, stop=True)
                avv = av[0:65].rearrange("p (j c) -> p j c", j=2)[:, :, 0:144]
                if pr % 2 == 0:
                    nc.scalar.copy(
                        attn_u[:, 2 * pr:2 * pr + 2, off:off + 144], avv)
                else:
                    nc.vector.tensor_copy(
                        attn_u[:, 2 * pr:2 * pr + 2, off:off + 144], avv)

        def phase_C_start(g, attn_u):
            sums_g = spool.tile([8, GTOK], F32, tag="sums", name=f"sums{g}")
            for h in range(H):
                nc.sync.dma_start(sums_g[h:h + 1, :], attn_u[64:65, h, :])
            lnsum = spool.tile([8, GTOK], F32, tag="lnsum", name=f"ln{g}")
            recip_g = spool.tile([8, GTOK], F32R, tag="recip", name=f"rc{g}")
            nc.scalar.activation(lnsum[:], sums_g[:], AF.Ln)
            nc.scalar.activation(recip_g[:], lnsum[:], AF.Exp, scale=-1.0)
            attn_c = acpool.tile([128, 4, GTOK], F32R, tag="ac", name=f"ac{g}")
            return recip_g, attn_c

        def phase_C_part(g, i, attn_u, recip_g, attn_c):
            g0 = g * GTOK
            NB = GTOK // 2  # 288
            if i < 2:
                nt = i
                for h in range(H):
                    fc, r0 = h // 2, (h % 2) * 64
                    bc = ps_pj.tile([64, NB], F32, tag="pj", name=f"pjb{g}_{i}")
                    nc.tensor.matmul(bc[:], e_all[:, 64 * h:64 * h + 64],
                                     recip_g[:, NB * nt:NB * nt + NB],
                                     start=True, stop=True)
                    nc.vector.tensor_mul(
                        attn_c[r0:r0 + 64, fc, NB * nt:NB * nt + NB],
                        attn_u[0:64, h, NB * nt:NB * nt + NB],
                        bc[:])
                return
            tlist = [0, 1, 2] if i == 2 else [3, 4]
            for t in tlist:
                rows = min(128, GTOK - 128 * t)
                p = ps_av.tile([128, D], F32, tag="av", name=f"avp{g}_{t}")
                for fc in range(4):
                    nc.tensor.matmul(
                        p[:rows, :],
                        attn_c[:, fc, 128 * t:128 * t + rows],
                        p_w_sb[:, fc, :],
                        start=(fc == 0), stop=(fc == 3))
                o_sb = opool.tile([128, D], F32, tag="osb", name=f"osb{g}_{t}")
                nc.vector.tensor_add(o_sb[:rows, :], p[:rows, :],
                                     bias_bc[:rows, :])
                nc.sync.dma_start(
                    out[g0 + 128 * t:g0 + 128 * t + rows, :], o_sb[:rows, :])

        # interleaved software pipeline:
        # slot g, part i: A(g) part i | C(g-2) part i | B(g-1) batch i
        state = {}
        au = {}
        cst = {}
        for g in range(NG + 2):
            if g < NG:
                state[g] = phase_A_load(g)
            if 0 <= g - 1 < NG:
                au[g - 1] = phase_B_start(g - 1)
            if 0 <= g - 2 < NG:
                cst[g - 2] = phase_C_start(g - 2, au[g - 2])
            for i in range(GB):
                if g < NG:
                    phase_A_part(g, i, state[g])
                if 0 <= g - 2 < NG:
                    phase_C_part(g - 2, i, au[g - 2], *cst[g - 2])
                if 0 <= g - 1 < NG:
                    phase_B_part(g - 1, i, state[g - 1], au[g - 1])
            state.pop(g - 2, None)
            au.pop(g - 3, None)
            cst.pop(g - 3, None)

    nc.compile()
    return nc


def _get_nc():
    if "nc" not in _CACHE:
        _CACHE["nc"] = build()
    return _CACHE["nc"]


def _prep_core_inputs(x, xk, kv_wT, q_wT, p_wT, p_b):
    """x, xk: [BPC, N, D] fp32 for one core."""
    def fm(a):  # [tok, D] -> [4, 128, tok] feature-major chunks
        t = np.ascontiguousarray(a.reshape(BPC * N, D).T)  # [D, tok]
        return t.reshape(4, 128, BPC * N)

    e_mat = np.zeros((8, D), dtype=np.float32)
    for h in range(H):
        e_mat[h, 64 * h:64 * h + 64] = 1.0
    return {
        "xT": fm(x), "xkT": fm(xk),
        "kv_wT": kv_wT, "q_wT": q_wT, "p_wT": p_wT, "p_b": p_b,
        "e_mat": e_mat,
    }


def kernel(x, topo_all_fea, kv_w, q_w, proj_w, proj_b, is_end):
    x = np.asarray(x, dtype=np.float32)
    topo = np.asarray(topo_all_fea, dtype=np.float32)
    kv_w = np.asarray(kv_w, dtype=np.float32)
    q_w = np.asarray(q_w, dtype=np.float32)
    proj_w = np.asarray(proj_w, dtype=np.float32)
    proj_b = np.asarray(proj_b, dtype=np.float32)
    end = bool(np.asarray(is_end).item()) if not isinstance(is_end, bool) \
        else is_end

    xk = x + topo if end else x

    kv_wT = np.ascontiguousarray(kv_w.T).reshape(4, 128, 2 * D)
    q_wT = np.ascontiguousarray(q_w.T * SCALE).reshape(4, 128, D)
    p_wT = np.ascontiguousarray(proj_w.T).reshape(4, 128, D)

    nc = _get_nc()
    in_maps = [
        _prep_core_inputs(x[c * BPC:(c + 1) * BPC],
                          xk[c * BPC:(c + 1) * BPC],
                          kv_wT, q_wT, p_wT, proj_b)
        for c in range(N_CORES)
    ]
    res = run_bass_kernel_spmd(nc, in_maps, core_ids=list(range(N_CORES)))
    outs = [res.results[c]["out"].reshape(BPC, N, D) for c in range(N_CORES)]
    return np.concatenate(outs, axis=0)
